# revision 2
# baseline (speedup 1.0000x reference)
"""Causal self-attention Trainium2 Bass kernel, data-parallel over 8 NeuronCores.

Problem (hardcoded): x [8, 2048, 1024] fp32; w_attn [1024, 3072]; b_attn [3072];
w_proj [1024, 1024]; b_proj [1024]. H=16 heads, D=64.

Row-tiered fp8 pipeline on top of the bf16 baseline: fp8e4m3 + DoubleRow
matmuls (2 contraction chunks per instruction) everywhere the causal
structure makes quantization noise harmless, bf16 where few-key attention
rows amplify it (error at row t scales ~1/sqrt(t)):
  - q/k production: rows < 256 bf16, rows >= 256 fp8 DoubleRow
    (wqk scaled x32 host-side in BOTH dtypes so exp scale stays uniform)
  - v production: key chunks < 4 bf16 (stored bf16 AND fp8), chunks >= 4
    fp8 DoubleRow (stored fp8)
  - attention PV: q-block 0 (rows < 512) bf16 with bf16 ex; blocks >= 1
    fp8: ex stored fp8e4m3, off-diagonal chunk pairs via DoubleRow,
    diagonal chunks as fp8 singles
  - output rows < 256: bf16 yn/yT/wp; rows >= 256: fp8 yn/yT, wp x16 fp8
    DoubleRow, the 1/16 unscale folded into the existing part copy /
    final add (scalar_tensor_tensor)
S = QK^T stays bf16 (contraction is 64 <= 128: fp8 cannot reduce its
streaming cycles). Numerics (numpy sim of exact quantization points):
rel_err 0.0039 vs all-bf16 baseline 0.0039.

Everything else (single fused pipeline, merged even/odd-head exp, softmax
denominator riding the PV matmul as a ones column, debt-paced feeder
generators keeping PE busy under the in-order queues) follows the bf16
baseline; see its docstring.
"""

import numpy as np
from contextlib import ExitStack

import ml_dtypes

import concourse.bacc as bacc
import concourse.tile as tile
from concourse import mybir
from concourse.bass_utils import run_bass_kernel_spmd

F32 = mybir.dt.float32
BF16 = mybir.dt.bfloat16
F8 = mybir.dt.float8e4
BF = ml_dtypes.bfloat16
F8NP = ml_dtypes.float8_e4m3fn
DR = mybir.MatmulPerfMode.DoubleRow
P = 128

SWQK = 32.0     # wqk host scale (both dtypes) -> folded into exp scale
SWP = 16.0      # wp fp8 host scale -> folded into part copy / final add
TQK = 256       # q/k rows below this produced in bf16
TPJ = 256       # output rows below this projected in bf16 (2 i-blocks)
NVB = 4         # v key chunks below this stored/produced in bf16

# cost-model cycle estimates (us) used only for emission pacing (REAL hw:
# fp8 DoubleRow is 1 cycle per moving row covering 2 contraction chunks)
_MM_US = 0.0004167      # PE cycle @2.4GHz
_ACT_US = 0.000833      # ACT cycle @1.2GHz


def build_program(T=2048, C=1024, H=16, n_cores=8,
                  with_bias_attn=False, with_bias_proj=False):
    D = C // H            # 64
    assert D == 64 and H % 2 == 0
    CIN = C // P          # 8 contraction chunks
    PAIRS = C // P        # 8 head pairs
    TKC = T // P          # 16 k-chunks
    QB = 512
    NQB = T // QB         # 4
    SUB = QB // P         # 4
    NI_B = TPJ // P       # bf16 output i-blocks (2)
    scale = (1.0 / float(np.sqrt(D))) / (SWQK * SWQK)

    nc = bacc.Bacc("TRN2", target_bir_lowering=False, debug=False,
                   num_devices=n_cores)

    xT_in = nc.dram_tensor("xT", [C, 512], BF16, kind="ExternalInput")
    xT8_in = nc.dram_tensor("xT8", [C, T], F8, kind="ExternalInput")
    wqk_in = nc.dram_tensor("wqk", [2 * C, C], BF16, kind="ExternalInput")
    wqk8_in = nc.dram_tensor("wqk8", [2 * C, C], F8, kind="ExternalInput")
    wv_in = nc.dram_tensor("wv", [C, C], BF16, kind="ExternalInput")
    wv8_in = nc.dram_tensor("wv8", [C, C], F8, kind="ExternalInput")
    wp_in = nc.dram_tensor("wp", [C, C], BF16, kind="ExternalInput")
    wp8_in = nc.dram_tensor("wp8", [C, C], F8, kind="ExternalInput")
    identb_in = nc.dram_tensor("identb", [P, P], BF16, kind="ExternalInput")
    ident8_in = nc.dram_tensor("ident8", [P, P], F8, kind="ExternalInput")
    mask_in = nc.dram_tensor("mask", [P, 2 * P], BF16, kind="ExternalInput")
    mask8_in = nc.dram_tensor("mask8", [P, 2 * P], F8, kind="ExternalInput")
    if with_bias_attn:
        b_attn = nc.dram_tensor("b_attn", [1, 3 * C], BF16,
                                kind="ExternalInput")
    if with_bias_proj:
        b_proj = nc.dram_tensor("b_proj", [1, C], BF16, kind="ExternalInput")
    out_d = nc.dram_tensor("out", [T, C], F32, kind="ExternalOutput")

    with tile.TileContext(nc) as tc, ExitStack() as ctx:
        ctx.enter_context(nc.allow_low_precision(reason="fp8/bf16 pipeline"))
        pool_c = ctx.enter_context(tc.tile_pool(name="const", bufs=1))
        identb_t = pool_c.tile([P, P], BF16, tag="identb")
        mask_t = pool_c.tile([P, 2 * P], BF16, tag="mask")
        mask8_t = pool_c.tile([P, 2 * P], F8, tag="mask8")
        # (their DMAs are emitted after the x head loads below — the DMA
        # queue issues in order and x gates the first matmuls)
        if with_bias_attn:
            ba_t = pool_c.tile([1, 3 * C], BF16, tag="ba")
            nc.sync.dma_start(ba_t[:], b_attn[:])
        if with_bias_proj:
            bp_t = pool_c.tile([1, C], BF16, tag="bp")
            nc.sync.dma_start(bp_t[:], b_proj[:])
        if with_bias_attn or with_bias_proj:
            ones_row = pool_c.tile([1, 512], BF16, tag="ones_row")
            nc.gpsimd.memset(ones_row[:], 1.0)

        # resident tensors (single tiles: one DMA each — the DMA queue
        # issues descriptors at ~650ns apiece, so count matters at startup)
        pool_xT = ctx.enter_context(tc.tile_pool(name="xT", bufs=1))
        # bf16 x is only read by the bf16 q/k rows [0:TQK) and the bf16 v
        # key chunks [0:512) — columns beyond 512 have no bf16 readers
        xT_t = pool_xT.tile([P, CIN, 512], BF16, tag="xT", name="xT")
        # fp8 x is only read for t >= TQK (q/k moving slices) and as lhsT
        # for key chunks >= NVB (t >= 512): store columns [TQK, T) only
        xT8_t = pool_xT.tile([P, CIN // 2, 2, T - TQK], F8, tag="xT8",
                             name="xT8")
        pool_v = ctx.enter_context(tc.tile_pool(name="vres", bufs=1))
        vb_t = [pool_v.tile([P, H, D + 1], BF16, tag=f"vb{i}", name=f"vb{i}")
                for i in range(NVB)]
        v8_t = [pool_v.tile([P, 2, H, D + 1], F8, tag=f"v8{i}", name=f"v8{i}")
                for i in range(TKC // 2)]
        pool_y = ctx.enter_context(tc.tile_pool(name="yres", bufs=1))
        yTb_sb = [pool_y.tile([P, TPJ], BF16, tag=f"yb{j}", name=f"yb{j}")
                  for j in range(CIN)]
        yT8_sb = [pool_y.tile([P, 2, T - TPJ], F8, tag=f"y8{j}", name=f"y8{j}")
                  for j in range(CIN // 2)]

        # working pools
        pool_qk = ctx.enter_context(tc.tile_pool(name="qkpool", bufs=8))
        pool_wqk = ctx.enter_context(tc.tile_pool(name="wqk", bufs=4))
        pool_wv = ctx.enter_context(tc.tile_pool(name="wv", bufs=1))
        pool_exb = ctx.enter_context(tc.tile_pool(name="exbpool", bufs=3))
        pool_ex8 = ctx.enter_context(tc.tile_pool(name="ex8pool", bufs=3))
        pool_yn = ctx.enter_context(tc.tile_pool(name="ynpool", bufs=6))
        pool_rc = ctx.enter_context(tc.tile_pool(name="rcpool", bufs=8))
        pool_ost = ctx.enter_context(tc.tile_pool(name="ostpool", bufs=3))
        pool_part = ctx.enter_context(tc.tile_pool(name="partpool", bufs=32))

        # PSUM: sT 2x2 banks + y 2x1 + mm 2x1 = 8 banks exactly
        psum_s = ctx.enter_context(
            tc.tile_pool(name="ps_s", bufs=2, space="PSUM"))
        psum_y = ctx.enter_context(
            tc.tile_pool(name="ps_y", bufs=2, space="PSUM"))
        psum_mm = ctx.enter_context(
            tc.tile_pool(name="ps_mm", bufs=2, space="PSUM"))

        for i in range(NVB):
            nc.gpsimd.memset(vb_t[i][:, :, D:D + 1], 1.0)
        for i in range(TKC // 2):
            nc.gpsimd.memset(v8_t[i][:, :, :, D:D + 1], 1.0)
        # split x loads: the head columns unblock the lead-in (first q/k
        # slice + v chunks 0-3) ~5x sooner; the xT8 tail is emitted after
        # the lead-in below and streams during pair 0's attention
        nc.sync.dma_start(xT_t[:],
                          xT_in[:].rearrange("(j p) t -> p j t", p=P))
        nc.sync.dma_start(
            xT8_t[:, :, :, 0:512 - TQK],
            xT8_in[:, TQK:512].rearrange("(jj i p) t -> p jj i t",
                                         i=2, p=P))
        nc.sync.dma_start(identb_t[:], identb_in[:])
        nc.sync.dma_start(mask_t[:], mask_in[:])
        nc.sync.dma_start(mask8_t[:], mask8_in[:])

        def emit_x_tails():
            # per-512-column pieces so tt-slice n only waits on its piece
            for t0 in range(512, T, 512):
                nc.sync.dma_start(
                    xT8_t[:, :, :, t0 - TQK:t0 - TQK + 512],
                    xT8_in[:, t0:t0 + 512].rearrange(
                        "(jj i p) t -> p jj i t", i=2, p=P))

        qkT = {}        # pr -> (qT tile, kT tile)
        parts = {}      # staged proj half-0 partial sums, keyed (i, g)
        state = {"v_prog": {0: -1, 1: -1}, "qk_prog": {}, "tr7": 0,
                 "tr": {}, "trset": {pr: set() for pr in range(PAIRS)},
                 "cur_pair": -1}

        # ---------------- feeder generators ----------------
        def gen_qkv(pr):
            # qT_pr reuses qT_{pr-4}'s SBUF slot (bufs=8), whose last reader
            # is attention(pr-4)'s S matmul: gate on attention progress to
            # avoid cycling the in-order DVE queue against PE.
            while state["cur_pair"] < pr - 2:
                yield None
            qt = pool_qk.tile([P, T], BF16, tag="qk", name=f"qT{pr}")
            kt = pool_qk.tile([P, T], BF16, tag="qk", name=f"kT{pr}")
            qkT[pr] = (qt, kt)
            state["qk_prog"][pr] = -1
            wmb = {}
            wm8 = {}
            for m in (pr, PAIRS + pr):
                wb = pool_wqk.tile([P, CIN, P], BF16, tag="wqk", name="wmb")
                nc.sync.dma_start(
                    wb[:],
                    wqk_in[m * P:(m + 1) * P, :].rearrange(
                        "p (j n) -> p j n", n=P))
                wmb[m] = wb
                w8 = pool_wqk.tile([P, CIN // 2, 2, P], F8, tag="wqk8",
                                   name="wm8")
                nc.sync.dma_start(
                    w8[:],
                    wqk8_in[m * P:(m + 1) * P, :].rearrange(
                        "p (jj i n) -> p jj i n", i=2, n=P))
                wm8[m] = w8
            # q/k interleaved per 512-wide t-slice so attention(pr) qb j can
            # start as soon as slices <= j exist
            for tt in range(T // 512):
                t0 = tt * 512
                for m, dst in ((pr, qt), (PAIRS + pr, kt)):
                    ps = psum_mm.tile([P, 512], F32, tag="mm", name="ps_qk")
                    if tt == 0:
                        # rows [0:TQK) bf16 (first accumulation group owns
                        # the bank's start=True)
                        for j in range(CIN):
                            nc.tensor.matmul(
                                ps[:, 0:TQK], wmb[m][:, j, :],
                                xT_t[:, j, 0:TQK],
                                start=(j == 0), stop=(j == CIN - 1),
                                skip_group_check=True)
                            if j == 3:
                                yield 0.45
                        # rows [TQK:512) fp8 DoubleRow (start=False: the
                        # bank's per-element bits were cleared above)
                        for jj in range(CIN // 2):
                            nc.tensor.matmul(
                                ps[:, TQK:512], wm8[m][:, jj],
                                xT8_t[:, jj, :, 0:512 - TQK],
                                start=False, stop=(jj == CIN // 2 - 1),
                                perf_mode=DR, skip_group_check=True)
                        if with_bias_attn:
                            col0 = m * P if m < PAIRS else C + (m - PAIRS) * P
                            nc.tensor.matmul(
                                ps[:], ba_t[0:1, col0:col0 + P],
                                ones_row[0:1, :], start=False, stop=True)
                        yield 0.45
                    else:
                        for jj in range(CIN // 2):
                            nc.tensor.matmul(
                                ps[:], wm8[m][:, jj],
                                xT8_t[:, jj, :, t0 - TQK:t0 - TQK + 512],
                                start=(jj == 0),
                                stop=(jj == CIN // 2 - 1
                                      and not with_bias_attn),
                                perf_mode=DR, skip_group_check=True)
                            if jj == 1:
                                yield 0.45
                        if with_bias_attn:
                            col0 = m * P if m < PAIRS else C + (m - PAIRS) * P
                            nc.tensor.matmul(
                                ps[:], ba_t[0:1, col0:col0 + P],
                                ones_row[0:1, :], start=False, stop=True)
                    nc.vector.tensor_copy(dst[:, t0:t0 + 512], ps[:])
                    yield 0.45
                state["qk_prog"][pr] = tt
        def gen_v(slab):
            # slab 1 feeds pairs 4-7 only: hold its emission back so it can
            # fill the late pairs' exp-latency gaps instead of the early ones
            while slab == 1 and state["cur_pair"] < 3:
                yield None
            g = slab * 512
            wvb = pool_wv.tile([P, CIN, 512], BF16, tag="wv", name="wvb")
            nc.sync.dma_start(
                wvb[:], wv_in[:, g:g + 512].rearrange("(j p) n -> p j n",
                                                      p=P))
            wv8 = pool_wv.tile([P, CIN // 2, 2, 512], F8, tag="wv8",
                               name="wv8")
            nc.sync.dma_start(
                wv8[:], wv8_in[:, g:g + 512].rearrange(
                    "(jj i p) n -> p jj i n", i=2, p=P))
            wvb_t = {(j, 0): wvb[:, j] for j in range(CIN)}
            wv8_t = {(jj, 0): wv8[:, jj] for jj in range(CIN // 2)}
            for i in range(TKC):
                ps = psum_mm.tile([P, 512], F32, tag="mm", name="ps_v")
                if i < NVB:
                    for j in range(CIN):
                        nc.tensor.matmul(
                            ps[:], xT_t[:, j, i * P:(i + 1) * P],
                            wvb_t[(j, 0)],
                            start=(j == 0),
                            stop=(j == CIN - 1 and not with_bias_attn),
                            skip_group_check=True)
                        if j == 3:
                            yield 0.9
                else:
                    for jj in range(CIN // 2):
                        nc.tensor.matmul(
                            ps[:],
                            xT8_t[:, jj, :, i * P - TQK:(i + 1) * P - TQK],
                            wv8_t[(jj, 0)],
                            start=(jj == 0),
                            stop=(jj == CIN // 2 - 1 and not with_bias_attn),
                            perf_mode=DR, skip_group_check=True)
                        if jj == 1:
                            yield 0.45
                if with_bias_attn:
                    nc.tensor.matmul(
                        ps[:], ones_row[0:1, 0:P],
                        ba_t[0:1, 2 * C + g:2 * C + g + 512],
                        start=False, stop=True)
                psh = ps[:].rearrange("p (h d) -> p h d", d=D)
                nc.vector.tensor_copy(
                    v8_t[i // 2][:, i % 2, g // D:(g + 512) // D, 0:D], psh)
                if i < NVB:
                    nc.vector.tensor_copy(
                        vb_t[i][:, g // D:(g + 512) // D, 0:D], psh)
                    yield 0.45
                state["v_prog"][slab] = i
                yield 0.45

        def gen_proj_half(h):
            # contraction split: half 0 (yT chunks 0-3) only needs pairs 0-3
            # and becomes PE fill for the otherwise-starved pairs 4-6; its
            # partial sums stage in SBUF (bf16) and half 1 adds them back.
            while state["cur_pair"] < (3 if h == 0 else PAIRS - 1):
                yield None
            js = list(range(4 * h, 4 * h + 4))
            jjs = [2 * h, 2 * h + 1]
            # half 1 reuses the (dead by then) wv slots: its gate releases
            # only after gen_v's last reader, and the Tile WAR tracking
            # orders the overwrite DMAs behind it. Sizes match exactly.
            wpb = pool_wv.tile([P, 2, 4, 512], BF16,
                               tag=("wpb0" if h == 0 else "wv"), name="wpb")
            nc.sync.dma_start(
                wpb[:],
                wp_in[4 * h * P:(4 * h + 4) * P, :].rearrange(
                    "(j p) (g n) -> p g j n", p=P, n=512))
            wp8 = pool_wv.tile([P, 2, 2, 2, 512], F8,
                               tag=("wp80" if h == 0 else "wv8"), name="wp8")
            nc.sync.dma_start(
                wp8[:],
                wp8_in[4 * h * P:(4 * h + 4) * P, :].rearrange(
                    "(jj i p) (g n) -> p g jj i n", i=2, p=P, n=512))
            wpb_t = {(j, g): wpb[:, g // 512, j - 4 * h]
                     for j in js for g in (0, 512)}
            wp8_t = {(jj, g): wp8[:, g // 512, jj - 2 * h]
                     for jj in jjs for g in (0, 512)}
            # i-outer: each transpose unlock frees BOTH g-slabs of block i
            for i in range(TKC):
                ost = None
                for g in (0, 512):
                    last_pr = 4 * h + 3
                    while (state["cur_pair"] <= last_pr and
                           i not in state["trset"][last_pr]):
                        yield None
                    ps = psum_mm.tile([P, 512], F32, tag="mm", name="ps_o")
                    if i < NI_B:
                        for j in js:
                            nc.tensor.matmul(
                                ps[:], yTb_sb[j][:, i * P:(i + 1) * P],
                                wpb_t[(j, g)],
                                start=(j == js[0]),
                                stop=(j == js[-1] and not
                                      (h == 1 and with_bias_proj)),
                                skip_group_check=True)
                            if j == js[1]:
                                yield 0.45
                    else:
                        c0 = i * P - TPJ
                        for jj in jjs:
                            nc.tensor.matmul(
                                ps[:], yT8_sb[jj][:, :, c0:c0 + P],
                                wp8_t[(jj, g)],
                                start=(jj == jjs[0]),
                                stop=(jj == jjs[-1] and not
                                      (h == 1 and with_bias_proj)),
                                perf_mode=DR, skip_group_check=True)
                        yield 0.45
                    if h == 0:
                        part = pool_part.tile([P, 512], BF16, tag="part",
                                              name="part")
                        if i < NI_B:
                            nc.vector.tensor_copy(part[:], ps[:])
                        else:
                            nc.vector.tensor_scalar(
                                part[:], ps[:], 1.0 / SWP, None,
                                op0=mybir.AluOpType.mult)
                        parts[(i, g)] = part
                    else:
                        if with_bias_proj:
                            nc.tensor.matmul(
                                ps[:], ones_row[0:1, 0:P],
                                bp_t[0:1, g:g + 512],
                                start=False, stop=True)
                        if ost is None:
                            ost = pool_ost.tile([P, 2, 512], F32, tag="ost",
                                                name="ost")
                        oslice = ost[:, g // 512]
                        if i < NI_B:
                            nc.vector.tensor_add(oslice, ps[:],
                                                 parts.pop((i, g))[:])
                        else:
                            nc.vector.scalar_tensor_tensor(
                                oslice, ps[:], 1.0 / SWP,
                                parts.pop((i, g))[:],
                                op0=mybir.AluOpType.mult,
                                op1=mybir.AluOpType.add)
                        if g == 512:
                            nc.sync.dma_start(
                                out_d[i * P:(i + 1) * P, :],
                                ost[:].rearrange("p g n -> p (g n)"))
                    yield 0.5

        class Feeder:
            def __init__(self):
                self.gens = []

            def push(self, g):
                self.gens.append(g)

            def pull_one(self):
                """Advance one unit from the first non-blocked generator
                (blocked heads are skipped, order otherwise preserved).
                Returns cost (us), 0.0 if all blocked, None if exhausted."""
                idx = 0
                pl = state.get("pulls", {}).get(state["cur_pair"])
                while idx < len(self.gens):
                    try:
                        cost = next(self.gens[idx])
                    except StopIteration:
                        self.gens.pop(idx)
                        continue
                    if cost is None:
                        idx += 1        # gated — try the next generator
                        continue
                    if pl is not None:
                        pl[0] += cost
                        pl[1] += 1
                    return cost
                if pl is not None:
                    pl[2] += 1
                return None if not self.gens else 0.0

            def drain(self):
                while self.pull_one() is not None:
                    pass

        feeder = Feeder()

        # ---------------- attention ----------------
        def attention(pr):
            state["pulls"] = state.get("pulls", {})
            state["pulls"][pr] = [0.0, 0, 0]   # us pulled, n pulled, n blocked
            state["cur_pair"] = pr
            qT, kT = qkT[pr]
            slab = pr // 4
            pending1 = []   # stage1: normalize (DVE)
            pending2 = []   # stage2: transpose + yT copy (PE+DVE)

            def emit_stage1(y_e, y_o, s, i):
                rc_e = pool_rc.tile([P, 1], F32, tag="rc", name="rc_e")
                rc_o = pool_rc.tile([P, 1], F32, tag="rc", name="rc_o")
                yn = pool_yn.tile([P, P], BF16, tag="yn", name="yn")
                with nc.allow_low_precision(reason="softmax normalize"):
                    nc.vector.reciprocal(
                        rc_e[:], y_e[:, 65 * s + D:65 * s + D + 1])
                    nc.vector.reciprocal(
                        rc_o[:], y_o[:, 65 * s + D:65 * s + D + 1])
                    nc.vector.tensor_scalar(
                        yn[:, 0:D], y_e[:, 65 * s:65 * s + D], rc_e[:, 0:1],
                        None, op0=mybir.AluOpType.mult)
                    nc.vector.tensor_scalar(
                        yn[:, D:2 * D], y_o[:, 65 * s:65 * s + D], rc_o[:, 0:1],
                        None, op0=mybir.AluOpType.mult)
                pending2.append((yn, i))

            def emit_stage2(yn, i):
                # PE transpose in bf16 (walrus rejects fp8 transpose
                # outputs); the fp8 conversion rides the DVE yT copy
                tr = psum_mm.tile([P, 512], BF16, tag="mm", name="tr")
                nc.tensor.transpose(tr[:, 0:P], yn[:], identb_t[:])
                if i < NI_B:
                    nc.vector.tensor_copy(yTb_sb[pr][:, i * P:(i + 1) * P],
                                          tr[:, 0:P])
                else:
                    c0 = i * P - TPJ
                    nc.vector.tensor_copy(
                        yT8_sb[pr // 2][:, pr % 2, c0:c0 + P], tr[:, 0:P])
                state["tr"][pr] = i + 1
                state["trset"][pr].add(i)
                if pr == PAIRS - 1:
                    state["tr7"] = i + 1

            debt = [0.0]

            def fill(extra=0.0):
                # in the last pair every pulled unit shortens the otherwise
                # ACT-idle projection tail: pull as hard as supply allows
                last = pr == PAIRS - 1
                debt[0] += extra + (0.25 if last else 0.0)
                pulls = 0
                while debt[0] > 0 and pulls < (4 if last else 2):
                    cost = feeder.pull_one()
                    if not cost:
                        break
                    debt[0] -= cost
                    pulls += 1
                debt[0] = min(debt[0], 8.0 if last else 4.0)

            for qb in range(NQB):
                q0 = qb * QB
                fp8 = qb > 0
                # this qb's q/k slices must already be emitted (the v
                # chunks are gated per-c below: PV(c) needs only chunk c)
                spins = 0
                while state["qk_prog"][pr] < qb:
                    c_ = feeder.pull_one()
                    spins += 1
                    if c_ is None or spins > 100000:
                        raise RuntimeError("feeder stuck before qk ready")
                y_e = psum_y.tile([P, 512], F32, tag="y", name="y_e")
                y_o = psum_y.tile([P, 512], F32, tag="y", name="y_o")
                started = [False]

                def emit_pv_b(ex, n0, c, y_e=y_e, y_o=y_o, qb=qb):
                    # bf16 singles (q-block 0): ex [P, 1024] bf16, vb chunks
                    s_min = max(0, c - 4 * qb)
                    for s in range(s_min, SUB):
                        # start=True only on the very first matmul into each
                        # y bank (it clears the whole bank; later regions
                        # first-write via the cleared per-element bits)
                        st_ = not started[0]
                        started[0] = True
                        sp_ = (c == 4 * qb + s)
                        nc.tensor.matmul(
                            y_e[:, 65 * s:65 * s + 65],
                            ex[:, s * P:(s + 1) * P],
                            vb_t[c][:, 2 * pr, :],
                            start=st_, stop=sp_, skip_group_check=True)
                        nc.tensor.matmul(
                            y_o[:, 65 * s:65 * s + 65],
                            ex[:, QB + s * P - n0:QB + (s + 1) * P - n0],
                            vb_t[c][:, 2 * pr + 1, :],
                            start=st_, stop=sp_, skip_group_check=True)
                        if sp_:
                            pending1.append((y_e, y_o, s, 4 * qb + s))

                def emit_pv_pair8(ex8, c, y_e=y_e, y_o=y_o, qb=qb):
                    # fp8 DoubleRow over the off-diagonal chunk pair
                    # (c-1, c): both n0 == 0
                    for s in range(SUB):
                        st_ = not started[0]
                        started[0] = True
                        nc.tensor.matmul(
                            y_e[:, 65 * s:65 * s + 65],
                            ex8[:, :, s * P:(s + 1) * P],
                            v8_t[c // 2][:, :, 2 * pr, :],
                            start=st_, stop=False,
                            perf_mode=DR, skip_group_check=True)
                        nc.tensor.matmul(
                            y_o[:, 65 * s:65 * s + 65],
                            ex8[:, :, QB + s * P:QB + (s + 1) * P],
                            v8_t[c // 2][:, :, 2 * pr + 1, :],
                            start=st_, stop=False,
                            perf_mode=DR, skip_group_check=True)

                def emit_pv_diag8(ex8, n0, c, y_e=y_e, y_o=y_o, qb=qb):
                    # fp8 singles for a diagonal chunk (slot c%2 of ex8)
                    s_min = max(0, c - 4 * qb)
                    for s in range(s_min, SUB):
                        st_ = not started[0]
                        started[0] = True
                        sp_ = (c == 4 * qb + s)
                        nc.tensor.matmul(
                            y_e[:, 65 * s:65 * s + 65],
                            ex8[:, c % 2, s * P:(s + 1) * P],
                            v8_t[c // 2][:, c % 2, 2 * pr, :],
                            start=st_, stop=sp_, skip_group_check=True)
                        nc.tensor.matmul(
                            y_o[:, 65 * s:65 * s + 65],
                            ex8[:, c % 2,
                                QB + s * P - n0:QB + (s + 1) * P - n0],
                            v8_t[c // 2][:, c % 2, 2 * pr + 1, :],
                            start=st_, stop=sp_, skip_group_check=True)
                        if sp_:
                            pending1.append((y_e, y_o, s, 4 * qb + s))

                pv_queue = []
                ex_pair = [None]
                for c in range(4 * qb + 4):
                    n0 = max(0, c * P - q0)
                    sT = psum_s.tile([P, 2 * QB], F32, tag="sT", name="sT")
                    nc.tensor.matmul(
                        sT[:, n0:QB],
                        kT[0:D, c * P:(c + 1) * P],
                        qT[0:D, q0 + n0:q0 + QB],
                        start=True, stop=True, tile_position=(0, 0))
                    nc.tensor.matmul(
                        sT[:, QB:2 * QB - n0],
                        kT[D:2 * D, c * P:(c + 1) * P],
                        qT[D:2 * D, q0 + n0:q0 + QB],
                        start=True, stop=True, tile_position=(D, 0))
                    if pending2:
                        emit_stage2(*pending2.pop(0))
                    if pending1:
                        emit_stage1(*pending1.pop(0))
                    if not fp8:
                        ex = pool_exb.tile([P, 2 * QB], BF16, tag="ex",
                                           name="ex")
                        exdst = ex[:, n0:2 * QB - n0]
                        mk_t = mask_t
                    else:
                        if c % 2 == 0:
                            ex_pair[0] = pool_ex8.tile(
                                [P, 2, 2 * QB], F8, tag="ex8", name="ex8")
                        ex = ex_pair[0]
                        exdst = ex[:, c % 2, n0:2 * QB - n0]
                        mk_t = mask8_t
                    nc.scalar.activation(exdst,
                                         sT[:, n0:2 * QB - n0],
                                         mybir.ActivationFunctionType.Exp,
                                         scale=scale)
                    if c * P >= q0:   # diagonal 128-blocks: causal mask
                        # one strided-AP multiply covers both heads' blocks
                        if not fp8:
                            blk = ex[:, n0:2 * QB - n0].rearrange(
                                "p (b s) -> p b s", b=2)
                        else:
                            blk = ex[:, c % 2, n0:2 * QB - n0].rearrange(
                                "p (b s) -> p b s", b=2)
                        mk = mk_t[:].rearrange("p (b s) -> p b s", b=2)
                        nc.vector.tensor_mul(blk[:, :, 0:P], blk[:, :, 0:P],
                                             mk[:, :, :])
                    # debt-paced feeder fill: keep PE busy while ACT exps
                    s_min = max(0, c - 4 * qb)
                    act_c = (2 * QB - 2 * n0) * _ACT_US + 0.21
                    if not fp8:
                        pe_c = (2 * (QB - n0) +
                                2 * (SUB - s_min) * 65) * _MM_US + 0.06
                    else:
                        # PV per chunk (amortized): off-diag DoubleRow pairs
                        # halve to 65*4 per chunk; diagonal singles keep
                        # 65*(SUB-s_min)*2
                        if c < 4 * qb:
                            pv_cyc = 65 * SUB
                        else:
                            pv_cyc = 65 * (SUB - s_min) * 2
                        pe_c = (2 * (QB - n0) + pv_cyc) * _MM_US + 0.06
                    fill(act_c - pe_c)
                    # software pipeline (depth 2): PV lags the S/exp front by
                    # two chunks so the PE never couples to exp completion
                    pv_queue.append((ex, n0, c))
                    if len(pv_queue) > 1:
                        it = pv_queue.pop(0)
                        spins = 0
                        while state["v_prog"][slab] < it[2]:
                            c_ = feeder.pull_one()
                            spins += 1
                            if c_ is None or spins > 100000:
                                raise RuntimeError("feeder stuck before v")
                        if not fp8:
                            emit_pv_b(*it)
                        else:
                            cc = it[2]
                            if cc < 4 * qb:      # off-diagonal
                                if cc % 2 == 1:
                                    emit_pv_pair8(it[0], cc)
                            else:
                                emit_pv_diag8(it[0], it[1], cc)
                for it in pv_queue:
                    spins = 0
                    while state["v_prog"][slab] < it[2]:
                        c_ = feeder.pull_one()
                        spins += 1
                        if c_ is None or spins > 100000:
                            raise RuntimeError("feeder stuck before v")
                    if not fp8:
                        emit_pv_b(*it)
                    else:
                        cc = it[2]
                        if cc < 4 * qb:
                            if cc % 2 == 1:
                                emit_pv_pair8(it[0], cc)
                        else:
                            emit_pv_diag8(it[0], it[1], cc)
                # the y_e/y_o PSUM slots are recycled by the next qb's
                # allocation: every pending normalize reading them must be
                # emitted before that (stage2 may stay pending)
                while pending1:
                    emit_stage1(*pending1.pop(0))
                    feeder.pull_one()
            # drain transposes, interleaving feeder units
            while pending2:
                emit_stage2(*pending2.pop(0))
                feeder.pull_one()

        # ---------------- program ----------------
        # lead-in: pair 0's first q/k slice + first 4 v chunks directly
        emit_x_tails()
        g0 = gen_qkv(0)
        while state["qk_prog"].get(0, -1) < 0:
            next(g0)
        gv0 = gen_v(0)
        while state["v_prog"][0] < 0:
            next(gv0)
        feeder.push(g0)
        feeder.push(gv0)
        feeder.push(gen_qkv(1))
        feeder.push(gen_qkv(2))
        feeder.push(gen_qkv(3))
        feeder.push(gen_v(1))
        feeder.push(gen_qkv(4))
        feeder.push(gen_qkv(5))
        feeder.push(gen_qkv(6))
        feeder.push(gen_qkv(7))
        feeder.push(gen_proj_half(0))
        feeder.push(gen_proj_half(1))

        for pr in range(PAIRS):
            # barrier: this pair's first q/k slice must be emitted
            spins = 0
            while state["qk_prog"].get(pr, -1) < 0:
                c_ = feeder.pull_one()
                spins += 1
                if c_ is None or spins > 100000:
                    raise RuntimeError("feeder stuck before qk ready")
            attention(pr)
        feeder.drain()
        import os
        if os.environ.get("FEED_DEBUG"):
            for k in sorted(state.get("pulls", {})):
                us, n, blocked = state["pulls"][k]
                print(f"pair {k}: pulled {us:.1f}us in {n} units, "
                      f"{blocked} dry pulls")

    nc.compile()
    return nc


def make_const_inputs():
    # S^T diagonal block mask: valid iff tq_local >= tk_local
    tri = np.triu(np.ones((P, P), np.float32))
    mask = np.concatenate([tri, tri], axis=1)
    return mask


def make_in_maps(inputs, n_cores=8):
    """Host-side marshalling: shard x over batch, dual-precision copies,
    pre-pack weight chunks into contiguous lhsT tiles."""
    x = np.asarray(inputs["x"], dtype=np.float32)
    w_attn = np.asarray(inputs["w_attn"], dtype=np.float32)
    w_proj = np.asarray(inputs["w_proj"], dtype=np.float32)
    b_attn = np.asarray(inputs.get("b_attn", 0), dtype=np.float32)
    b_proj = np.asarray(inputs.get("b_proj", 0), dtype=np.float32)
    B, T, C = x.shape

    wqk = w_attn[:, :2 * C] * SWQK      # [C, 2C], scaled in BOTH dtypes
    # bf16 chunk m tile [p, j*128+n] = wqk[j*128+p, m*128+n]
    wqk_packed = np.ascontiguousarray(
        wqk.reshape(C // P, P, 2 * C // P, P)     # [j, p, m, n]
        .transpose(2, 1, 0, 3)                    # [m, p, j, n]
        .reshape(2 * C, C)).astype(BF)
    # fp8 chunk m tile [p, jj*256 + i*128 + n] = wqk[(2jj+i)*128+p, m*128+n]
    wqk8_packed = np.ascontiguousarray(
        wqk.reshape(C // (2 * P), 2, P, 2 * C // P, P)  # [jj, i, p, m, n]
        .transpose(3, 2, 0, 1, 4)                       # [m, p, jj, i, n]
        .reshape(2 * C, C)).astype(F8NP)
    wv = np.ascontiguousarray(w_attn[:, 2 * C:])
    wp = np.ascontiguousarray(w_proj)
    mask = make_const_inputs()
    ident = np.eye(P, dtype=np.float32)

    wba = bool(np.any(b_attn != 0))
    wbp = bool(np.any(b_proj != 0))
    in_maps = []
    for i in range(n_cores):
        xT = np.ascontiguousarray(x[i].T)
        m = {"xT": np.ascontiguousarray(xT[:, :512]).astype(BF),
             "xT8": xT.astype(F8NP),
             "wqk": wqk_packed, "wqk8": wqk8_packed,
             "wv": wv.astype(BF), "wv8": wv.astype(F8NP),
             "wp": wp.astype(BF), "wp8": (wp * SWP).astype(F8NP),
             "identb": ident.astype(BF), "ident8": ident.astype(F8NP),
             "mask": mask.astype(BF), "mask8": mask.astype(F8NP)}
        if wba:
            m["b_attn"] = b_attn.reshape(1, -1).astype(BF)
        if wbp:
            m["b_proj"] = b_proj.reshape(1, -1).astype(BF)
        in_maps.append(m)
    return in_maps


_CACHE = {}


def _get_program(T, C, H, wba, wbp, n_cores):
    key = (T, C, H, wba, wbp, n_cores)
    if key not in _CACHE:
        _CACHE[key] = build_program(T=T, C=C, H=H, n_cores=n_cores,
                                    with_bias_attn=wba, with_bias_proj=wbp)
    return _CACHE[key]


def kernel(x, w_attn, b_attn, w_proj, b_proj):
    x = np.asarray(x, dtype=np.float32)
    B, T, C = x.shape
    H = 16
    n_cores = 8
    assert B == n_cores

    inputs = {"x": x, "w_attn": w_attn, "b_attn": b_attn,
              "w_proj": w_proj, "b_proj": b_proj}
    in_maps = make_in_maps(inputs, n_cores)
    wba = "b_attn" in in_maps[0]
    wbp = "b_proj" in in_maps[0]
    nc = _get_program(T, C, H, wba, wbp, n_cores)

    res = run_bass_kernel_spmd(nc, in_maps, list(range(n_cores)))
    return np.stack([res.results[i]["out"] for i in range(n_cores)], axis=0)
